# revision 1
# baseline (speedup 1.0000x reference)
"""CrossModalGraphLayer on 8 Trainium2 NeuronCores (Bass/Tile).

Computation (reference):
    proj  = input @ W1.T                                  [N, D]
    msg   = edge_val[:, None] * proj[edge_col]            [E, D]
    neigh = segment_sum(msg, edge_row, N)                 [N, D]
    h     = concat([input + neigh, input * neigh], -1)    [N, 2D]
    out   = leaky_relu(h @ W2.T, 0.01)                    [N, D]

Strategy (SPMD, one program on 8 cores; per-core data differs):
  - Destination rows sharded: core m owns rows [m*12500, (m+1)*12500).
  - Phase A: every core computes the full proj table (bf16) into its own
    DRAM via PE matmuls (stationary = inputT node-block, moving = W1T).
  - Phase B: edges (host-sorted by dest row, bucketed by 512-row dest
    block x source-quarter for int16 gather indices) are gathered 128 at
    a time with dma_gather ([128 edges, 128 feat] tiles), then a tiny
    host-built scatter matrix S [128, 16] turns segment-sum into PE
    matmuls accumulating neighT [feat, rows] in a PSUM bank (dynamic
    column windows; per-bank zero-matmul clears has_written).
  - Phase C (inline per block): a = x + neigh, b = x * neigh on DVE
    (PSUM operand), yT = W2aT.T@a + W2bT.T@b on PE, leaky-relu on ACT,
    DMA out. Host reassembles/transposes.
"""
import numpy as np
import ml_dtypes

import concourse.bass as bass
import concourse.tile as tile
from concourse import bacc, mybir
from concourse.bass_utils import run_bass_kernel_spmd
from concourse.tile_rust import add_dep_helper

N = 100000
E = 1600000
D = 128
NCORES = 8
RPC = N // NCORES        # rows per core = 12500
RB = 512                 # rows per destination block (one PSUM bank)
NBLK = (RPC + RB - 1) // RB   # 25 (24 full + 212)
NQ = 4                   # source quarters (int16 idx limit)
QROWS = N // NQ          # 25000
NPAD = ((N + 127) // 128) * 128  # proj table rows padded to 128 multiple
SW = 64                  # scatter-matrix width (rows per matmul window)
                         # (quarter-split -> ~4 edges/row/group -> 128 edges
                         #  span ~32 rows; 64 gives an ~8-sigma margin)
NEG_SLOPE = 0.01

DEBUG_SKIP = set()  # subsets of {"gather", "mm", "vload", "sdma"} for HW bisect

bf16 = ml_dtypes.bfloat16


# ---------------------------------------------------------------- host side

def preprocess(edge_row, edge_col, edge_val):
    """Sort/bucket edges; build per-core gather-idx, S, bases arrays.

    Returns (meta, per_core) where meta has the SPMD-uniform structure
    (budgets per (block, quarter)) and per_core the per-core data arrays.
    """
    core = edge_row // RPC
    lrow = edge_row - core * RPC             # 0..RPC-1
    blk = lrow // RB                         # 0..NBLK-1
    brow = (lrow - blk * RB).astype(np.int32)  # 0..RB-1
    q = edge_col // QROWS
    lcol = (edge_col - q * QROWS).astype(np.int16)

    gid = (core * NBLK + blk) * NQ + q       # global group id
    order = np.argsort(gid.astype(np.int64) * RB + brow, kind="stable")
    gid_s = gid[order]
    brow_s = brow[order]
    lcol_s = lcol[order]
    val_s = edge_val[order]

    ngroups = NCORES * NBLK * NQ
    counts = np.bincount(gid_s, minlength=ngroups).reshape(NCORES, NBLK, NQ)
    budget = (counts.max(axis=0) + 127) // 128 * 128      # [NBLK, NQ]
    ntiles_g = budget // 128                               # [NBLK, NQ]
    NT = int(ntiles_g.sum())
    NIDX = NT * 128

    g_off = np.zeros(NBLK * NQ + 1, np.int64)              # slot offset per group
    g_off[1:] = np.cumsum(budget.reshape(-1))
    estart = np.zeros(ngroups + 1, np.int64)
    estart[1:] = np.cumsum(counts.reshape(-1))

    t_off_g = np.zeros(NBLK * NQ + 1, np.int64)            # tile offset per group
    t_off_g[1:] = np.cumsum(ntiles_g.reshape(-1))
    t_off_b = np.zeros(NBLK + 1, np.int64)                 # tile offset per block
    t_off_b[1:] = np.cumsum(ntiles_g.sum(axis=1))

    tt = np.arange(NIDX) // 128
    kk = np.arange(NIDX) % 128

    per_core = []
    for m in range(NCORES):
        idx_local = np.zeros(NIDX, np.int16)
        srow = np.zeros(NIDX, np.int32)
        sval = np.zeros(NIDX, np.float32)
        for g in range(NBLK * NQ):
            e0, e1 = estart[m * NBLK * NQ + g], estart[m * NBLK * NQ + g + 1]
            n = e1 - e0
            s0 = g_off[g]
            if n:
                idx_local[s0:s0 + n] = lcol_s[e0:e1]
                srow[s0:s0 + n] = brow_s[e0:e1]
                sval[s0:s0 + n] = val_s[e0:e1]
                # pad rows copy the last real row so tile spans stay tight
                srow[s0 + n:g_off[g + 1]] = brow_s[e1 - 1]
        sr2 = srow.reshape(NT, 128)
        tmin = sr2.min(axis=1)
        tmax = sr2.max(axis=1)
        assert (tmax - tmin < SW).all(), "tile row-span exceeded SW"
        base = np.minimum(tmin, RB - SW).astype(np.int32)
        S = np.zeros((NT, 128, SW), np.float32)
        S[tt, kk, srow - base[tt]] = sval
        # wrapped idx layout [16, NIDX/16] replicated to 128 partitions
        idx_wrapped = np.tile(idx_local.reshape(NIDX // 16, 16).T, (8, 1))
        per_core.append(dict(
            idx=np.ascontiguousarray(idx_wrapped),
            s=np.ascontiguousarray(S.transpose(1, 0, 2).astype(bf16)),
            bases=base.reshape(1, NT),
        ))
    meta = dict(NT=NT, NIDX=NIDX,
                ntiles=ntiles_g, t_off_b=t_off_b, g_off=g_off, t_off_g=t_off_g)
    return meta, per_core


# -------------------------------------------------------------- device side

def build_program(meta, K=1, phases=("proj", "edge", "final"), timing_mode=False):
    NT, NIDX = meta["NT"], meta["NIDX"]
    ntiles = meta["ntiles"]
    t_off_b = meta["t_off_b"]
    g_off = meta["g_off"]

    nc = bacc.Bacc("TRN2", target_bir_lowering=False, debug=False,
                   num_devices=NCORES)
    dt = mybir.dt
    # timing_mode: value-independent tensors become internal (garbage) DRAM
    # so repeated timing calls don't pay ~500MB of host->device staging.
    big = dict(kind="ExternalInput") if not timing_mode else {}
    xtbf = nc.dram_tensor("xtbf", [128, NPAD], dt.bfloat16, **big).ap()
    xtf = nc.dram_tensor("xtf", [128, RPC], dt.float32, **big).ap()
    w1t_d = nc.dram_tensor("w1t", [128, 128], dt.bfloat16, **big).ap()
    w2at_d = nc.dram_tensor("w2at", [128, 128], dt.bfloat16, **big).ap()
    w2bt_d = nc.dram_tensor("w2bt", [128, 128], dt.bfloat16, **big).ap()
    idx_d = nc.dram_tensor("idxs", [128, NIDX // 16], dt.int16, kind="ExternalInput").ap()
    s_d = nc.dram_tensor("sdat", [128, NT, SW], dt.bfloat16, **big).ap()
    bases_d = nc.dram_tensor("bases", [1, NT], dt.int32, kind="ExternalInput").ap()
    proj_d = nc.dram_tensor("proj", [NPAD, 128], dt.bfloat16).ap()
    yt_d = nc.dram_tensor("yt", [128, RPC], dt.float32, kind="ExternalOutput").ap()

    NSB = NPAD // 128 // 16 + (1 if (NPAD // 128) % 16 else 0)  # proj super-blocks

    with tile.TileContext(nc) as tc:
        def body(_i, ctx_pools):
            consts = ctx_pools
            w1 = consts.tile([128, 128], dt.bfloat16)
            nc.sync.dma_start(w1[:], w1t_d[:])
            w2a = consts.tile([128, 128], dt.bfloat16)
            nc.sync.dma_start(w2a[:], w2at_d[:])
            w2b = consts.tile([128, 128], dt.bfloat16)
            nc.sync.dma_start(w2b[:], w2bt_d[:])
            zt = consts.tile([1, 512], dt.bfloat16)
            nc.vector.memset(zt[:], 0.0)
            bases_t = consts.tile([1, NT], dt.int32)
            nc.sync.dma_start(bases_t[:], bases_d[:])
            idx_t = consts.tile([128, NIDX // 16], dt.int16)
            nc.sync.dma_start(idx_t[:], idx_d[:])

            proj_writes = []
            # ---------------- Phase A: proj table ----------------
            if "proj" in phases:
                nblocks_total = NPAD // 128  # 782
                with tc.tile_pool(name="pa_x", bufs=3) as xp, \
                     tc.tile_pool(name="pa_st", bufs=2) as stp, \
                     tc.tile_pool(name="pa_ps", bufs=2, space="PSUM") as pp:
                    blk0 = 0
                    while blk0 < nblocks_total:
                        nb = min(16, nblocks_total - blk0)
                        n0 = blk0 * 128
                        nn = nb * 128
                        xt = xp.tile([128, nn], dt.bfloat16, tag="pa_x")
                        nc.sync.dma_start(xt[:], xtbf[:, n0:n0 + nn])
                        ps = pp.tile([128, nn], dt.float32, tag="pa_ps")
                        for i in range(nb):
                            nc.tensor.matmul(
                                ps[:, i * 128:(i + 1) * 128],
                                xt[:, i * 128:(i + 1) * 128],
                                w1[:],
                                start=True, stop=True, skip_group_check=True)
                        stage = stp.tile([128, nn], dt.bfloat16, tag="pa_st")
                        nc.vector.tensor_copy(stage[:], ps[:])
                        wi = nc.sync.dma_start(
                            proj_d[n0:n0 + nn, :].rearrange("(b p) d -> p b d", p=128),
                            stage[:].rearrange("p (b d) -> p b d", d=128))
                        proj_writes.append(wi)
                        blk0 += nb

            # ---------------- Phase B + C: edges + output ----------------
            if "edge" in phases:
                first_gather = [None]
                with tc.tile_pool(name="eb_g", bufs=3) as gp, \
                     tc.tile_pool(name="eb_s", bufs=2) as sp, \
                     tc.tile_pool(name="eb_ps", bufs=2, space="PSUM") as ep, \
                     tc.tile_pool(name="fc_x", bufs=2) as fxp, \
                     tc.tile_pool(name="fc_ab", bufs=4) as fab, \
                     tc.tile_pool(name="fc_ps", bufs=2, space="PSUM") as fyp, \
                     tc.tile_pool(name="fc_y", bufs=2) as fyo:
                    for b in range(NBLK):
                        rb0 = b * RB
                        rbn = min(RB, RPC - rb0)
                        nt_blk = int(t_off_b[b + 1] - t_off_b[b])
                        st = sp.tile([128, max(nt_blk, 1), SW], dt.bfloat16, tag="eb_s")
                        if nt_blk and "sdma" not in DEBUG_SKIP:
                            nc.sync.dma_start(
                                st[:, :nt_blk, :],
                                s_d[:, int(t_off_b[b]):int(t_off_b[b + 1]), :])
                        ps = ep.tile([128, 512], dt.float32, tag="eb_ps")
                        nc.tensor.matmul(ps[:, 0:512], zt[0:1, 0:128], zt[0:1, 0:512],
                                         start=True, stop=False, skip_group_check=True)
                        tloc = 0
                        n_done = 0
                        base_vals = []
                        for q in range(NQ):
                            nt_g = int(ntiles[b][q])
                            if nt_g == 0:
                                continue
                            g = b * NQ + q
                            s0 = int(g_off[g])
                            ni = nt_g * 128
                            gbuf = gp.tile([128, nt_g, 128], dt.bfloat16, tag="eb_g")
                            if "gather" not in DEBUG_SKIP:
                                gi = nc.gpsimd.dma_gather(
                                    gbuf[:],
                                    proj_d[q * QROWS:min((q + 1) * QROWS, NPAD), :],
                                    idx_t[:, s0 // 16:(s0 + ni) // 16],
                                    ni, ni, 128,
                                    single_packet=False)
                                if first_gather[0] is None:
                                    first_gather[0] = gi
                            for c in range(nt_g):
                                if "vload" not in DEBUG_SKIP:
                                    if not base_vals:
                                        t0 = int(t_off_b[b]) + tloc
                                        nload = min(8, nt_blk - tloc)
                                        _, vals = nc.values_load_multi_w_load_instructions(
                                            bases_t[0:1, t0:t0 + nload],
                                            engines=[mybir.EngineType.PE],
                                            min_val=0, max_val=RB - SW,
                                            skip_runtime_bounds_check=True)
                                        base_vals = list(vals)
                                    bv = base_vals.pop(0)
                                else:
                                    bv = 0
                                n_done += 1
                                if "mm" not in DEBUG_SKIP:
                                    nc.tensor.matmul(
                                        ps[:, bass.ds(bv, SW)] if "vload" not in DEBUG_SKIP else ps[:, 0:SW],
                                        gbuf[:, c, :],
                                        st[:, tloc, :],
                                        start=False, stop=(n_done == nt_blk),
                                        skip_group_check=True)
                                tloc += 1
                        # -------- Phase C inline --------
                        if "final" in phases:
                            xin = fxp.tile([128, rbn], dt.float32, tag="fc_x")
                            nc.sync.dma_start(xin[:], xtf[:, rb0:rb0 + rbn])
                            at = fab.tile([128, rbn], dt.bfloat16, tag="fc_a")
                            nc.vector.tensor_add(at[:], ps[:, :rbn], xin[:])
                            bt = fab.tile([128, rbn], dt.bfloat16, tag="fc_b")
                            nc.vector.tensor_mul(bt[:], ps[:, :rbn], xin[:])
                            yp = fyp.tile([128, rbn], dt.float32, tag="fc_ps")
                            nc.tensor.matmul(yp[:], w2a[:], at[:],
                                             start=True, stop=False,
                                             skip_group_check=True)
                            nc.tensor.matmul(yp[:], w2b[:], bt[:],
                                             start=False, stop=True,
                                             skip_group_check=True)
                            # leaky_relu(x) = max(x, alpha*x)
                            ya = fyo.tile([128, rbn], dt.float32, tag="fc_ya")
                            nc.scalar.mul(ya[:], yp[:], NEG_SLOPE)
                            yo = fyo.tile([128, rbn], dt.float32, tag="fc_y")
                            nc.vector.tensor_max(yo[:], yp[:], ya[:])
                            nc.sync.dma_start(yt_d[:, rb0:rb0 + rbn], yo[:])
                # explicit ordering: gathers read proj_d written by phase A
                if proj_writes and first_gather[0] is not None:
                    for wi in proj_writes:
                        add_dep_helper(first_gather[0].ins, wi.ins,
                                       sync=True, reason="gather waits on proj writes")

        with tc.tile_pool(name="consts", bufs=1) as consts:
            if K == 1:
                body(0, consts)
            else:
                with tc.For_i(0, K, 1) as i:
                    body(i, consts)

    nc.compile()
    return nc


# ------------------------------------------------------------------ driver

def make_inputs(input, edge_row, edge_col, edge_val, W1, W2):
    input = np.asarray(input, np.float32)
    meta, per_core = preprocess(np.asarray(edge_row), np.asarray(edge_col),
                                np.asarray(edge_val, np.float32))
    xT = np.ascontiguousarray(input.T)                        # [128, N]
    xtbf = np.zeros((128, NPAD), bf16)
    xtbf[:, :N] = xT.astype(bf16)
    w1t = np.ascontiguousarray(np.asarray(W1, np.float32).T.astype(bf16))
    W2 = np.asarray(W2, np.float32)
    w2at = np.ascontiguousarray(W2[:, :128].T.astype(bf16))
    w2bt = np.ascontiguousarray(W2[:, 128:].T.astype(bf16))
    in_maps = []
    for m in range(NCORES):
        in_maps.append(dict(
            xtbf=xtbf,
            xtf=np.ascontiguousarray(xT[:, m * RPC:(m + 1) * RPC]),
            w1t=w1t, w2at=w2at, w2bt=w2bt,
            idxs=per_core[m]["idx"],
            sdat=per_core[m]["s"],
            bases=per_core[m]["bases"].astype(np.int32),
        ))
    return meta, in_maps


def kernel(input, edge_row, edge_col, edge_val, W1, W2):
    meta, in_maps = make_inputs(input, edge_row, edge_col, edge_val, W1, W2)
    nc = build_program(meta, K=1)
    res = run_bass_kernel_spmd(nc, in_maps, list(range(NCORES)))
    out = np.concatenate(
        [res.results[m]["yt"].T for m in range(NCORES)], axis=0)
    return np.ascontiguousarray(out.astype(np.float32))

